# revision 1
# baseline (speedup 1.0000x reference)
"""nn_MultiHeadAttention TRN2 kernel: 8-core tensor-parallel (2 heads/core).

Self-contained: builds and compiles the Bass/Tile SPMD program on first call,
shards the full inputs per-core on the host, runs via run_bass_kernel_spmd,
and concatenates the per-core sequence-block outputs into the full output.

Algorithm (per core, 2 heads of 16, head_dim 64, S=4096, D=1024):
  - feature-major layout: xT [D,S]; q/k projected with RoPE-permuted,
    transposed weight shards so rotary becomes a contiguous split-half
    rotation; v seq-major with a ones column (softmax denominator).
  - flash attention on transposed score tiles scoresT[j,i]: PE matmuls
    (heads row-packed), causal mask added as a -400 triangle on diagonal
    tiles (DVE), exp on ScalarE grouped 3 key-tiles per instruction,
    PV accumulates outT[65,512] in PSUM (row 64 = denominator).
  - normalize via DVE reciprocal + K=1 matmul partition-broadcast.
  - AllToAll re-shards from head-split to sequence-split; final projection
    against full Wo.T; each core emits out[512, 1024] f32.
"""

from contextlib import ExitStack

import numpy as np
import ml_dtypes

import concourse.tile as tile
from concourse import bacc, mybir
from concourse.bass_utils import run_bass_kernel_spmd

F32 = mybir.dt.float32
BF16 = mybir.dt.bfloat16

S = 4096
D = 1024
HD = 64
N_CORES = 8
KT = 128
BQ = 512


def _build():
    CHUNK = S // N_CORES
    n_qb = S // BQ
    bq = BQ
    n_kt = S // KT
    n_ft = D // 128

    nc = bacc.Bacc("TRN2", target_bir_lowering=False, debug=False, num_devices=N_CORES)

    xT = nc.dram_tensor("xT", [D, S], BF16, kind="ExternalInput")
    wq = nc.dram_tensor("wq", [D, 128], BF16, kind="ExternalInput")
    wk = nc.dram_tensor("wk", [D, 128], BF16, kind="ExternalInput")
    wv = nc.dram_tensor("wv", [D, 128], BF16, kind="ExternalInput")
    wo = nc.dram_tensor("wo", [D, D], BF16, kind="ExternalInput")
    cosP = nc.dram_tensor("cosP", [128, S], BF16, kind="ExternalInput")
    sinN = nc.dram_tensor("sinN", [128, S], BF16, kind="ExternalInput")
    lu = nc.dram_tensor("lu", [128, 128], BF16, kind="ExternalInput")
    out = nc.dram_tensor("out", [CHUNK, D], F32, kind="ExternalOutput")

    a2a_in = nc.dram_tensor("a2a_in", [N_CORES * 128, CHUNK], BF16)
    a2a_out = nc.dram_tensor("a2a_out", [N_CORES * 128, CHUNK], BF16)

    with tile.TileContext(nc) as tc, ExitStack() as ctx:
        sb = ctx.enter_context(tc.tile_pool(name="sb", bufs=1))
        xt_s = [sb.tile([128, S], BF16, tag=f"xt{t}", name=f"xt{t}") for t in range(n_ft)]
        wq_s = sb.tile([128, n_ft * 128], BF16, tag="wq", name="wq_s")
        wk_s = sb.tile([128, n_ft * 128], BF16, tag="wk", name="wk_s")
        wv_s = sb.tile([128, n_ft * 128], BF16, tag="wv", name="wv_s")
        wo_s = [sb.tile([128, D], BF16, tag=f"wo{t}", name=f"wo_s{t}") for t in range(n_ft)]
        cos_s = sb.tile([128, S], BF16, tag="cos", name="cos_s")
        sin_s = sb.tile([128, S], BF16, tag="sin", name="sin_s")
        lu_s = sb.tile([128, 128], BF16, tag="lu", name="lu_s")
        qA = sb.tile([128, S], BF16, tag="qA", name="qA")
        kA = sb.tile([128, S], BF16, tag="kA", name="kA")
        qB = sb.tile([128, S], BF16, tag="qB", name="qB")
        kB = sb.tile([128, S], BF16, tag="kB", name="kB")
        qT = sb.tile([128, S], BF16, tag="qT", name="qT")
        kT_ = sb.tile([128, S], BF16, tag="kT", name="kT_")
        v_aug = sb.tile([128, n_kt * 130], BF16, tag="vaug", name="v_aug")
        attnT = sb.tile([128, S], BF16, tag="attnT", name="attnT")
        aT = [sb.tile([128, CHUNK], BF16, tag=f"aT{t}", name=f"aT{t}") for t in range(n_ft)]

        for t in range(n_ft):
            nc.sync.dma_start(xt_s[t][:], xT[128 * t : 128 * (t + 1), :])
            nc.sync.dma_start(wq_s[:, 128 * t : 128 * (t + 1)], wq[128 * t : 128 * (t + 1), :])
            nc.sync.dma_start(wk_s[:, 128 * t : 128 * (t + 1)], wk[128 * t : 128 * (t + 1), :])
            nc.sync.dma_start(wv_s[:, 128 * t : 128 * (t + 1)], wv[128 * t : 128 * (t + 1), :])
            nc.sync.dma_start(wo_s[t][:], wo[128 * t : 128 * (t + 1), :])
        nc.sync.dma_start(cos_s[:], cosP[:, :])
        nc.sync.dma_start(sin_s[:], sinN[:, :])
        nc.sync.dma_start(lu_s[:], lu[:, :])

        psc = ctx.enter_context(tc.tile_pool(name="psc", bufs=2, space="PSUM"))
        ppv = ctx.enter_context(tc.tile_pool(name="ppv", bufs=2, space="PSUM"))

        # projections
        for w_s, dst in ((wq_s, qA), (wk_s, kA)):
            for nb in range(S // bq):
                p = psc.tile([128, bq], F32, tag="sc", name="p_qk")
                for t in range(n_ft):
                    nc.tensor.matmul(
                        p[:],
                        w_s[:, 128 * t : 128 * (t + 1)],
                        xt_s[t][:, bq * nb : bq * (nb + 1)],
                        start=(t == 0),
                        stop=(t == n_ft - 1),
                    )
                nc.scalar.copy(dst[:, bq * nb : bq * (nb + 1)], p[:])
        for st in range(n_kt):
            p = ppv.tile([128, 128], F32, tag="pv", name="p_v")
            for t in range(n_ft):
                nc.tensor.matmul(
                    p[:],
                    xt_s[t][:, 128 * st : 128 * (st + 1)],
                    wv_s[:, 128 * t : 128 * (t + 1)],
                    start=(t == 0),
                    stop=(t == n_ft - 1),
                )
            base = 130 * st
            nc.vector.tensor_copy(v_aug[:, base : base + 64], p[:, 0:64])
            nc.vector.tensor_copy(v_aug[:, base + 65 : base + 129], p[:, 64:128])
            nc.vector.memset(v_aug[:, base + 64 : base + 65], 1.0)
            nc.vector.memset(v_aug[:, base + 129 : base + 130], 1.0)

        # RoPE
        for A, B in ((qA, qB), (kA, kB)):
            for h in range(2):
                b0 = 64 * h
                nc.sync.dma_start(B[b0 : b0 + 32, :], A[b0 + 32 : b0 + 64, :])
                nc.sync.dma_start(B[b0 + 32 : b0 + 64, :], A[b0 : b0 + 32, :])
        for A, B, Rt in ((qA, qB, qT), (kA, kB, kT_)):
            nc.vector.tensor_mul(Rt[:], A[:], cos_s[:])
            nc.vector.tensor_mul(B[:], B[:], sin_s[:])
            nc.vector.tensor_add(Rt[:], Rt[:], B[:])

        # attention
        ones_col = sb.tile([1, 64], F32, tag="ones_col", name="ones_col")
        nc.vector.memset(ones_col[:], 1.0)
        GROUP = 3
        for Q in range(n_qb):
            q0 = bq * Q
            n_jt = min((q0 + bq) // KT, n_kt)
            outT = {}
            for h in range(2):
                outT[h] = ppv.tile([65, bq], F32, tag="pv", name=f"outT_h{h}")
            for h in range(2):
                hb = 64 * h
                jts = list(range(n_jt))
                groups = [jts[i : i + GROUP] for i in range(0, n_jt, GROUP)]
                for g in groups:
                    sc = psc.tile([128, len(g) * bq], F32, tag="sc", name="sc_g")
                    for idx, jt in enumerate(g):
                        nc.tensor.matmul(
                            sc[:, bq * idx : bq * (idx + 1)],
                            kT_[hb : hb + 64, KT * jt : KT * (jt + 1)],
                            qT[hb : hb + 64, q0 : q0 + bq],
                            start=True,
                            stop=True,
                        )
                        if KT * jt >= q0:
                            trim = KT * jt - q0
                            nc.vector.tensor_add(
                                sc[:, bq * idx + trim : bq * idx + trim + 128],
                                sc[:, bq * idx + trim : bq * idx + trim + 128],
                                lu_s[:],
                            )
                    expT = sb.tile([128, GROUP * bq], BF16, tag="expT", name="expT", bufs=2)
                    nc.scalar.activation(
                        expT[:, 0 : len(g) * bq],
                        sc[:],
                        mybir.ActivationFunctionType.Exp,
                        scale=0.125,
                    )
                    for idx, jt in enumerate(g):
                        trim = max(0, KT * jt - q0)
                        nc.tensor.matmul(
                            outT[h][:, trim:bq],
                            v_aug[:, 130 * jt : 130 * jt + 65]
                            if h == 0
                            else v_aug[:, 130 * jt + 65 : 130 * jt + 130],
                            expT[:, bq * idx + trim : bq * (idx + 1)],
                            start=(jt == 0),
                            stop=(jt == n_jt - 1),
                        )
                den_r = sb.tile([1, bq], F32, tag="den", name="den_r")
                nc.vector.reciprocal(den_r[:], outT[h][64:65, :])
                bc = psc.tile([64, bq], F32, tag="sc", name="bc")
                nc.tensor.matmul(bc[:], ones_col[:], den_r[:], start=True, stop=True)
                bc_sb = sb.tile([64, bq], F32, tag="bc_sb", name="bc_sb")
                nc.vector.tensor_copy(bc_sb[:], bc[:])
                nc.vector.tensor_mul(
                    attnT[hb : hb + 64, q0 : q0 + bq], outT[h][0:64, :], bc_sb[:]
                )

        # all-to-all: head-split -> sequence-split
        for j in range(N_CORES):
            nc.sync.dma_start(
                a2a_in[128 * j : 128 * (j + 1), :], attnT[:, CHUNK * j : CHUNK * (j + 1)]
            )
        nc.gpsimd.collective_compute(
            "AllToAll",
            mybir.AluOpType.bypass,
            replica_groups=[list(range(N_CORES))],
            ins=[a2a_in.ap().opt()],
            outs=[a2a_out.ap().opt()],
        )
        for t in range(n_ft):
            nc.sync.dma_start(aT[t][:], a2a_out[128 * t : 128 * (t + 1), :])

        # output projection
        for it in range(CHUNK // 128):
            for oh in range(D // 512):
                p = psc.tile([128, 512], F32, tag="sc", name="p_o")
                for t in range(n_ft):
                    nc.tensor.matmul(
                        p[:],
                        aT[t][:, 128 * it : 128 * (it + 1)],
                        wo_s[t][:, 512 * oh : 512 * (oh + 1)],
                        start=(t == 0),
                        stop=(t == n_ft - 1),
                    )
                ot = sb.tile([128, 512], F32, tag="oflush", name="ot")
                nc.scalar.copy(ot[:], p[:])
                nc.sync.dma_start(
                    out[128 * it : 128 * (it + 1), 512 * oh : 512 * (oh + 1)], ot[:]
                )

    nc.compile()
    return nc


def _host_prep(x, Wq, Wk, Wv, Wo):
    bf = ml_dtypes.bfloat16
    perm = np.empty(HD, dtype=np.int64)
    perm[:32] = np.arange(0, 64, 2)
    perm[32:] = np.arange(1, 64, 2)

    inv_freq = 1.0 / (10000.0 ** (np.arange(0, HD, 2, dtype=np.float32) / HD))
    fr = np.outer(np.arange(S, dtype=np.float32), inv_freq)
    cosA = np.cos(fr).T
    sinA = np.sin(fr).T
    cosP = np.tile(np.concatenate([cosA, cosA], 0), (2, 1)).astype(bf)
    sinN = np.tile(np.concatenate([-sinA, sinA], 0), (2, 1)).astype(bf)
    lu = np.tril(np.full((128, 128), -400.0, np.float32), k=-1).astype(bf)

    xT = np.ascontiguousarray(x.reshape(S, D).T).astype(bf)
    woT = np.ascontiguousarray(np.asarray(Wo, np.float32).T).astype(bf)

    in_maps = []
    for c in range(N_CORES):
        rows = np.concatenate([128 * c + 64 * h + perm for h in range(2)])
        in_maps.append(
            {
                "xT": xT,
                "wq": np.ascontiguousarray(np.asarray(Wq, np.float32)[rows].T).astype(bf),
                "wk": np.ascontiguousarray(np.asarray(Wk, np.float32)[rows].T).astype(bf),
                "wv": np.ascontiguousarray(
                    np.asarray(Wv, np.float32)[128 * c : 128 * (c + 1)].T
                ).astype(bf),
                "wo": woT,
                "cosP": cosP,
                "sinN": sinN,
                "lu": lu,
            }
        )
    return in_maps


_NC_CACHE = None


def kernel(x, Wq, Wk, Wv, Wo):
    global _NC_CACHE
    if _NC_CACHE is None:
        _NC_CACHE = _build()
    nc = _NC_CACHE
    in_maps = _host_prep(
        np.asarray(x, np.float32),
        np.asarray(Wq, np.float32),
        np.asarray(Wk, np.float32),
        np.asarray(Wv, np.float32),
        np.asarray(Wo, np.float32),
    )
    res = run_bass_kernel_spmd(nc, in_maps, core_ids=list(range(N_CORES)))
    full = np.concatenate([res.results[c]["out"] for c in range(N_CORES)], axis=0)
    return full.reshape(1, S, D).astype(np.float32)



# revision 4
# speedup vs baseline: 1.1128x; 1.1128x over previous
"""nn_MultiHeadAttention TRN2 kernel: 8-core tensor-parallel (2 heads/core).

Self-contained: builds and compiles the Bass/Tile SPMD program on first call,
shards the full inputs per-core on the host, runs via run_bass_kernel_spmd,
and concatenates the per-core sequence-block outputs into the full output.

v2 design (per core, 2 heads of 16, head_dim 64, S=4096, D=1024):
  - feature-major xT [D,S]; q/k projected with RoPE-permuted transposed
    weight shards so rotary is a contiguous split-half rotation; v seq-major
    into a 4-D v_aug tile [128, 32, 2, 65] whose 65th column is 1.0
    (softmax denominator via the PV matmul).
  - projection and attention emission interleaved per 512-block so the PE
    pipeline never drains (DVFS: sustained activity ramps PE 1.2->2.4 GHz).
  - flash attention on transposed score tiles: per Q-block the two heads'
    exp-groups are interleaved and PV emission is skewed one group behind
    scores, keeping TensorE busy while ScalarE computes exp.
  - normalization decoupled from the PSUM critical path: outT is copied to
    SBUF immediately; reciprocal_approx_fast + ones-column broadcast matmul
    + in-place multiply run lazily off the PE critical path.
  - per-Q-block staging DMAs feed one AllToAll (head-split -> seq-split);
    final projection against full Wo.T; each core emits out[512, 1024] f32.
"""

from contextlib import ExitStack

import numpy as np
import ml_dtypes

import concourse.tile as tile
from concourse import bacc, mybir
from concourse.bass_utils import run_bass_kernel_spmd

F32 = mybir.dt.float32
BF16 = mybir.dt.bfloat16

S = 4096
D = 1024
HD = 64
N_CORES = 8
KT = 128
BQ = 512
CHUNK = S // N_CORES
NFT = D // 128
NKT = S // KT
NQB = S // BQ
G = 3


def _build():
    nc = bacc.Bacc("TRN2", target_bir_lowering=False, debug=False, num_devices=N_CORES)

    xT = nc.dram_tensor("xT", [D, S], BF16, kind="ExternalInput")
    wq = nc.dram_tensor("wq", [D, 128], BF16, kind="ExternalInput")
    wk = nc.dram_tensor("wk", [D, 128], BF16, kind="ExternalInput")
    wv = nc.dram_tensor("wv", [D, 128], BF16, kind="ExternalInput")
    wo = nc.dram_tensor("wo", [D, D], BF16, kind="ExternalInput")
    cosP = nc.dram_tensor("cosP", [128, S], BF16, kind="ExternalInput")
    sinN = nc.dram_tensor("sinN", [128, S], BF16, kind="ExternalInput")
    lu = nc.dram_tensor("lu", [128, 128], BF16, kind="ExternalInput")
    out = nc.dram_tensor("out", [CHUNK, D], F32, kind="ExternalOutput")

    a2a_in = nc.dram_tensor("a2a_in", [N_CORES * 128, CHUNK], BF16)
    a2a_out = nc.dram_tensor("a2a_out", [N_CORES * 128, CHUNK], BF16)

    with tile.TileContext(nc) as tc, ExitStack() as ctx:
        sb = ctx.enter_context(tc.tile_pool(name="sb", bufs=1))
        xt_s = [sb.tile([128, S], BF16, tag=f"xt{t}", name=f"xt{t}") for t in range(NFT)]
        wq_s = sb.tile([128, NFT * 128], BF16, tag="wq", name="wq_s")
        wk_s = sb.tile([128, NFT * 128], BF16, tag="wk", name="wk_s")
        wv_s = sb.tile([128, NFT * 128], BF16, tag="wv", name="wv_s")
        wo_s = [sb.tile([128, D], BF16, tag=f"wo{t}", name=f"wo_s{t}") for t in range(NFT)]
        cos_s = sb.tile([128, S], BF16, tag="cos", name="cos_s")
        sin_s = sb.tile([128, S], BF16, tag="sin", name="sin_s")
        lu_s = sb.tile([128, 128], BF16, tag="lu", name="lu_s")
        qA = sb.tile([128, S], BF16, tag="qA", name="qA")
        kA = sb.tile([128, S], BF16, tag="kA", name="kA")
        qB = sb.tile([128, S], BF16, tag="qB", name="qB")
        kB = sb.tile([128, S], BF16, tag="kB", name="kB")
        qT = sb.tile([128, S], BF16, tag="qT", name="qT")
        kT_ = sb.tile([128, S], BF16, tag="kT", name="kT_")
        v_aug = sb.tile([128, NKT, 2, 65], BF16, tag="vaug", name="v_aug")
        attnT = sb.tile([128, S], BF16, tag="attnT", name="attnT")
        aT = [sb.tile([128, CHUNK], BF16, tag=f"aT{t}", name=f"aT{t}") for t in range(NFT)]
        ones_c = sb.tile([1, 64], BF16, tag="ones_c", name="ones_c")

        nc.vector.memset(v_aug[:], 1.0)
        nc.vector.memset(ones_c[:], 1.0)

        for t in range(NFT):
            r = slice(128 * t, 128 * (t + 1))
            nc.sync.dma_start(wq_s[:, r], wq[r, :])
            nc.sync.dma_start(wk_s[:, r], wk[r, :])
            nc.sync.dma_start(wv_s[:, r], wv[r, :])
            nc.sync.dma_start(wo_s[t][:], wo[r, :])
        nc.sync.dma_start(cos_s[:], cosP[:, :])
        nc.sync.dma_start(sin_s[:], sinN[:, :])
        nc.sync.dma_start(lu_s[:], lu[:, :])
        for nb in range(NQB):
            c = slice(BQ * nb, BQ * (nb + 1))
            for t in range(NFT):
                nc.sync.dma_start(xt_s[t][:, c], xT[128 * t : 128 * (t + 1), c])

        psc = ctx.enter_context(tc.tile_pool(name="psc", bufs=2, space="PSUM"))
        ppv = ctx.enter_context(tc.tile_pool(name="ppv", bufs=2, space="PSUM"))

        def proj_block(nb):
            c = slice(BQ * nb, BQ * (nb + 1))
            qp = psc.tile([128, BQ], F32, tag="sc", name="qp")
            for t in range(NFT):
                nc.tensor.matmul(
                    qp[:], wq_s[:, 128 * t : 128 * (t + 1)], xt_s[t][:, c],
                    start=(t == 0), stop=(t == NFT - 1),
                )
            kp = psc.tile([128, BQ], F32, tag="sc", name="kp")
            for t in range(NFT):
                nc.tensor.matmul(
                    kp[:], wk_s[:, 128 * t : 128 * (t + 1)], xt_s[t][:, c],
                    start=(t == 0), stop=(t == NFT - 1),
                )
            nc.vector.tensor_copy(qA[:, c], qp[:])
            nc.vector.tensor_copy(kA[:, c], kp[:])
            vp = psc.tile([128, BQ], F32, tag="sc", name="vp")
            for u in range(4):
                st = slice(BQ * nb + 128 * u, BQ * nb + 128 * (u + 1))
                for t in range(NFT):
                    nc.tensor.matmul(
                        vp[:, 128 * u : 128 * (u + 1)], xt_s[t][:, st],
                        wv_s[:, 128 * t : 128 * (t + 1)],
                        start=(t == 0), stop=(t == NFT - 1),
                    )
            nc.vector.tensor_copy(
                v_aug[:, 4 * nb : 4 * (nb + 1), :, 0:64],
                vp[:].rearrange("p (u h c) -> p u h c", u=4, h=2, c=64),
            )
            for A, B in ((qA, qB), (kA, kB)):
                for h in range(2):
                    b0 = 64 * h
                    nc.sync.dma_start(B[b0 : b0 + 32, c], A[b0 + 32 : b0 + 64, c])
                    nc.sync.dma_start(B[b0 + 32 : b0 + 64, c], A[b0 : b0 + 32, c])
            for A, B, R in ((qA, qB, qT), (kA, kB, kT_)):
                nc.vector.tensor_mul(R[:, c], A[:, c], cos_s[:, c])
                nc.vector.tensor_mul(B[:, c], B[:, c], sin_s[:, c])
                nc.vector.tensor_add(R[:, c], R[:, c], B[:, c])

        def attn_block(Q):
            q0 = BQ * Q
            n_jt = 4 * (Q + 1)
            jts = list(range(n_jt))
            groups = [jts[i : i + G] for i in range(0, n_jt, G)]
            seq = [(h, g) for h in range(2) for g in groups]
            outT = {}

            def emit_pv(h, g, ex):
                hb = 64 * h
                if g[0] == 0:
                    outT[h] = ppv.tile([65, BQ], F32, tag="pv", name=f"outT{h}")
                for idx, jt in enumerate(g):
                    trim = max(0, KT * jt - q0)
                    nc.tensor.matmul(
                        outT[h][:, trim:BQ],
                        v_aug[:, jt, h, :],
                        ex[:, BQ * idx + trim : BQ * (idx + 1)],
                        start=(jt == 0),
                        stop=(jt == n_jt - 1),
                    )

            pending = None
            for h, g in seq:
                hb = 64 * h
                sc = psc.tile([128, BQ * len(g)], F32, tag="sc", name="sc")
                for idx, jt in enumerate(g):
                    nc.tensor.matmul(
                        sc[:, BQ * idx : BQ * (idx + 1)],
                        kT_[hb : hb + 64, KT * jt : KT * (jt + 1)],
                        qT[hb : hb + 64, q0 : q0 + BQ],
                        start=True, stop=True,
                    )
                for idx, jt in enumerate(g):
                    if KT * jt >= q0:
                        trim = KT * jt - q0
                        nc.vector.tensor_add(
                            sc[:, BQ * idx + trim : BQ * idx + trim + 128],
                            sc[:, BQ * idx + trim : BQ * idx + trim + 128],
                            lu_s[:],
                        )
                ex = sb.tile([128, G * BQ], BF16, tag="expT", name="expT", bufs=3)
                nc.scalar.activation(
                    ex[:, 0 : BQ * len(g)], sc[:],
                    mybir.ActivationFunctionType.Exp, scale=0.125,
                )
                if pending is not None:
                    emit_pv(*pending)
                pending = (h, g, ex)
            emit_pv(*pending)

            for h in range(2):
                hb = 64 * h
                den_f = sb.tile([1, BQ], F32, tag=f"denf{h}", name=f"denf{h}", bufs=2)
                den_b = sb.tile([1, BQ], BF16, tag=f"denb{h}", name=f"denb{h}", bufs=2)
                nc.vector.reciprocal(den_f[:], outT[h][64:65, :])
                nc.vector.tensor_copy(attnT[hb : hb + 64, q0 : q0 + BQ], outT[h][0:64, :])
                nc.vector.tensor_copy(den_b[:], den_f[:])
                bc = psc.tile([64, BQ], F32, tag="sc", name="bc")
                nc.tensor.matmul(bc[:], ones_c[:], den_b[:], start=True, stop=True)
                nc.vector.tensor_mul(
                    attnT[hb : hb + 64, q0 : q0 + BQ],
                    attnT[hb : hb + 64, q0 : q0 + BQ],
                    bc[:],
                )
            nc.sync.dma_start(
                a2a_in[128 * Q : 128 * (Q + 1), :], attnT[:, q0 : q0 + BQ]
            )

        for nb in range(NQB):
            proj_block(nb)
            if nb >= 1:
                attn_block(nb - 1)
        attn_block(NQB - 1)

        nc.gpsimd.collective_compute(
            "AllToAll",
            mybir.AluOpType.bypass,
            replica_groups=[list(range(N_CORES))],
            ins=[a2a_in.ap().opt()],
            outs=[a2a_out.ap().opt()],
        )
        for t in range(NFT):
            nc.sync.dma_start(aT[t][:], a2a_out[128 * t : 128 * (t + 1), :])

        for it in range(CHUNK // 128):
            for oh in range(D // 512):
                p = psc.tile([128, 512], F32, tag="sc", name="p_o")
                for t in range(NFT):
                    nc.tensor.matmul(
                        p[:],
                        aT[t][:, 128 * it : 128 * (it + 1)],
                        wo_s[t][:, 512 * oh : 512 * (oh + 1)],
                        start=(t == 0), stop=(t == NFT - 1),
                    )
                ot = sb.tile([128, 512], F32, tag="oflush", name="ot", bufs=2)
                nc.scalar.copy(ot[:], p[:])
                nc.sync.dma_start(
                    out[128 * it : 128 * (it + 1), 512 * oh : 512 * (oh + 1)], ot[:]
                )

    nc.compile()
    return nc


def _host_prep(x, Wq, Wk, Wv, Wo):
    bf = ml_dtypes.bfloat16
    perm = np.empty(HD, dtype=np.int64)
    perm[:32] = np.arange(0, 64, 2)
    perm[32:] = np.arange(1, 64, 2)

    inv_freq = 1.0 / (10000.0 ** (np.arange(0, HD, 2, dtype=np.float32) / HD))
    fr = np.outer(np.arange(S, dtype=np.float32), inv_freq)
    cosA = np.cos(fr).T
    sinA = np.sin(fr).T
    cosP = np.tile(np.concatenate([cosA, cosA], 0), (2, 1)).astype(bf)
    sinN = np.tile(np.concatenate([-sinA, sinA], 0), (2, 1)).astype(bf)
    lu = np.tril(np.full((128, 128), -400.0, np.float32), k=-1).astype(bf)

    xT = np.ascontiguousarray(x.reshape(S, D).T).astype(bf)
    woT = np.ascontiguousarray(np.asarray(Wo, np.float32).T).astype(bf)

    in_maps = []
    for c in range(N_CORES):
        rows = np.concatenate([128 * c + 64 * h + perm for h in range(2)])
        in_maps.append(
            {
                "xT": xT,
                "wq": np.ascontiguousarray(np.asarray(Wq, np.float32)[rows].T).astype(bf),
                "wk": np.ascontiguousarray(np.asarray(Wk, np.float32)[rows].T).astype(bf),
                "wv": np.ascontiguousarray(
                    np.asarray(Wv, np.float32)[128 * c : 128 * (c + 1)].T
                ).astype(bf),
                "wo": woT,
                "cosP": cosP,
                "sinN": sinN,
                "lu": lu,
            }
        )
    return in_maps


_NC_CACHE = None


def kernel(x, Wq, Wk, Wv, Wo):
    global _NC_CACHE
    if _NC_CACHE is None:
        _NC_CACHE = _build()
    nc = _NC_CACHE
    in_maps = _host_prep(
        np.asarray(x, np.float32),
        np.asarray(Wq, np.float32),
        np.asarray(Wk, np.float32),
        np.asarray(Wv, np.float32),
        np.asarray(Wo, np.float32),
    )
    res = run_bass_kernel_spmd(nc, in_maps, core_ids=list(range(N_CORES)))
    full = np.concatenate([res.results[c]["out"] for c in range(N_CORES)], axis=0)
    return full.reshape(1, S, D).astype(np.float32)


# revision 8
# speedup vs baseline: 1.1491x; 1.0326x over previous
"""nn_MultiHeadAttention TRN2 kernel: 8-core tensor-parallel (2 heads/core).

Self-contained: builds and compiles the Bass/Tile SPMD program on first call,
shards the full inputs per-core on the host, runs via run_bass_kernel_spmd,
and concatenates the per-core sequence-block outputs into the full output.

v2 design (per core, 2 heads of 16, head_dim 64, S=4096, D=1024):
  - feature-major xT [D,S]; q/k projected with RoPE-permuted transposed
    weight shards so rotary is a contiguous split-half rotation; v seq-major
    into a 4-D v_aug tile [128, 32, 2, 65] whose 65th column is 1.0
    (softmax denominator via the PV matmul).
  - projection and attention emission interleaved per 512-block so the PE
    pipeline never drains (DVFS: sustained activity ramps PE 1.2->2.4 GHz).
  - flash attention on transposed score tiles: per Q-block the two heads'
    exp-groups are interleaved and PV emission is skewed one group behind
    scores, keeping TensorE busy while ScalarE computes exp.
  - normalization decoupled from the PSUM critical path: outT is copied to
    SBUF immediately; reciprocal_approx_fast + ones-column broadcast matmul
    + in-place multiply run lazily off the PE critical path.
  - per-Q-block staging DMAs feed one AllToAll (head-split -> seq-split);
    final projection against full Wo.T; each core emits out[512, 1024] f32.
"""

from contextlib import ExitStack

import numpy as np
import ml_dtypes

import concourse.tile as tile
from concourse import bacc, mybir
from concourse.bass_utils import run_bass_kernel_spmd

F32 = mybir.dt.float32
BF16 = mybir.dt.bfloat16

S = 4096
D = 1024
HD = 64
N_CORES = 8
KT = 128
BQ = 512
CHUNK = S // N_CORES
NFT = D // 128
NKT = S // KT
NQB = S // BQ
G = 3


def _build():
    nc = bacc.Bacc("TRN2", target_bir_lowering=False, debug=False, num_devices=N_CORES)

    xT = nc.dram_tensor("xT", [D, S], BF16, kind="ExternalInput")
    wq = nc.dram_tensor("wq", [D, 128], BF16, kind="ExternalInput")
    wk = nc.dram_tensor("wk", [D, 128], BF16, kind="ExternalInput")
    wv = nc.dram_tensor("wv", [D, 128], BF16, kind="ExternalInput")
    wo = nc.dram_tensor("wo", [D, D], BF16, kind="ExternalInput")
    cosP = nc.dram_tensor("cosP", [128, S], BF16, kind="ExternalInput")
    sinN = nc.dram_tensor("sinN", [128, S], BF16, kind="ExternalInput")
    lu = nc.dram_tensor("lu", [128, 128], BF16, kind="ExternalInput")
    out = nc.dram_tensor("out", [CHUNK, D], F32, kind="ExternalOutput")

    a2a_in = nc.dram_tensor("a2a_in", [N_CORES * 128, CHUNK], BF16)
    a2a_out = nc.dram_tensor("a2a_out", [N_CORES * 128, CHUNK], BF16)

    with tile.TileContext(nc) as tc, ExitStack() as ctx:
        sb = ctx.enter_context(tc.tile_pool(name="sb", bufs=1))
        xt_s = [sb.tile([128, S], BF16, tag=f"xt{t}", name=f"xt{t}") for t in range(NFT)]
        wq_s = sb.tile([128, NFT * 128], BF16, tag="wq", name="wq_s")
        wk_s = sb.tile([128, NFT * 128], BF16, tag="wk", name="wk_s")
        wv_s = sb.tile([128, NFT * 128], BF16, tag="wv", name="wv_s")
        wo_s = [sb.tile([128, D], BF16, tag=f"wo{t}", name=f"wo_s{t}") for t in range(NFT)]
        cos_s = sb.tile([128, S], BF16, tag="cos", name="cos_s")
        sin_s = sb.tile([128, S], BF16, tag="sin", name="sin_s")
        lu_s = sb.tile([128, 128], BF16, tag="lu", name="lu_s")
        qA = sb.tile([128, S], BF16, tag="qA", name="qA")
        kA = sb.tile([128, S], BF16, tag="kA", name="kA")
        qB = sb.tile([128, S], BF16, tag="qB", name="qB")
        kB = sb.tile([128, S], BF16, tag="kB", name="kB")
        qT = sb.tile([128, S], BF16, tag="qT", name="qT")
        kT_ = sb.tile([128, S], BF16, tag="kT", name="kT_")
        v_aug = sb.tile([128, NKT, 2, 65], BF16, tag="vaug", name="v_aug")
        attnT = sb.tile([128, S], BF16, tag="attnT", name="attnT")
        aT = [sb.tile([128, CHUNK], BF16, tag=f"aT{t}", name=f"aT{t}") for t in range(NFT)]
        ones_c = sb.tile([1, 64], BF16, tag="ones_c", name="ones_c")

        nc.vector.memset(v_aug[:], 1.0)
        nc.vector.memset(ones_c[:], 1.0)

        for t in range(NFT):
            r = slice(128 * t, 128 * (t + 1))
            nc.sync.dma_start(wq_s[:, r], wq[r, :])
            nc.sync.dma_start(wk_s[:, r], wk[r, :])
            nc.sync.dma_start(wv_s[:, r], wv[r, :])
            nc.sync.dma_start(wo_s[t][:], wo[r, :])
        nc.sync.dma_start(cos_s[:], cosP[:, :])
        nc.sync.dma_start(sin_s[:], sinN[:, :])
        nc.sync.dma_start(lu_s[:], lu[:, :])
        for nb in range(NQB):
            c = slice(BQ * nb, BQ * (nb + 1))
            for t in range(NFT):
                nc.sync.dma_start(xt_s[t][:, c], xT[128 * t : 128 * (t + 1), c])

        psc = ctx.enter_context(tc.tile_pool(name="psc", bufs=2, space="PSUM"))
        ppv = ctx.enter_context(tc.tile_pool(name="ppv", bufs=2, space="PSUM"))

        def proj_block(nb):
            c = slice(BQ * nb, BQ * (nb + 1))
            qp = psc.tile([128, BQ], F32, tag="sc", name="qp")
            for t in range(NFT):
                nc.tensor.matmul(
                    qp[:], wq_s[:, 128 * t : 128 * (t + 1)], xt_s[t][:, c],
                    start=(t == 0), stop=(t == NFT - 1),
                )
            kp = psc.tile([128, BQ], F32, tag="sc", name="kp")
            for t in range(NFT):
                nc.tensor.matmul(
                    kp[:], wk_s[:, 128 * t : 128 * (t + 1)], xt_s[t][:, c],
                    start=(t == 0), stop=(t == NFT - 1),
                )
            nc.vector.tensor_copy(qA[:, c], qp[:])
            nc.vector.tensor_copy(kA[:, c], kp[:])
            vp = psc.tile([128, BQ], F32, tag="sc", name="vp")
            for u in range(4):
                st = slice(BQ * nb + 128 * u, BQ * nb + 128 * (u + 1))
                for t in range(NFT):
                    nc.tensor.matmul(
                        vp[:, 128 * u : 128 * (u + 1)], xt_s[t][:, st],
                        wv_s[:, 128 * t : 128 * (t + 1)],
                        start=(t == 0), stop=(t == NFT - 1),
                    )
            nc.vector.tensor_copy(
                v_aug[:, 4 * nb : 4 * (nb + 1), :, 0:64],
                vp[:].rearrange("p (u h c) -> p u h c", u=4, h=2, c=64),
            )
            for A, B in ((qA, qB), (kA, kB)):
                for h in range(2):
                    b0 = 64 * h
                    nc.sync.dma_start(B[b0 : b0 + 32, c], A[b0 + 32 : b0 + 64, c])
                    nc.sync.dma_start(B[b0 + 32 : b0 + 64, c], A[b0 : b0 + 32, c])
            for A, B, R in ((qA, qB, qT), (kA, kB, kT_)):
                nc.vector.tensor_mul(R[:, c], A[:, c], cos_s[:, c])
                nc.vector.tensor_mul(B[:, c], B[:, c], sin_s[:, c])
                nc.vector.tensor_add(R[:, c], R[:, c], B[:, c])

        def attn_block(Q):
            q0 = BQ * Q
            n_jt = 4 * (Q + 1)
            jts = list(range(n_jt))
            groups = [jts[i : i + G] for i in range(0, n_jt, G)]
            seq = [(h, g) for h in range(2) for g in groups]
            outT = {}

            def emit_pv(h, g, ex):
                hb = 64 * h
                if g[0] == 0:
                    outT[h] = ppv.tile([65, BQ], F32, tag="pv", name=f"outT{h}")
                for idx, jt in enumerate(g):
                    trim = max(0, KT * jt - q0)
                    nc.tensor.matmul(
                        outT[h][:, trim:BQ],
                        v_aug[:, jt, h, :],
                        ex[:, BQ * idx + trim : BQ * (idx + 1)],
                        start=(jt == 0),
                        stop=(jt == n_jt - 1),
                    )

            pending = None
            for h, g in seq:
                hb = 64 * h
                sc = psc.tile([128, BQ * len(g)], F32, tag="sc", name="sc")
                for idx, jt in enumerate(g):
                    nc.tensor.matmul(
                        sc[:, BQ * idx : BQ * (idx + 1)],
                        kT_[hb : hb + 64, KT * jt : KT * (jt + 1)],
                        qT[hb : hb + 64, q0 : q0 + BQ],
                        start=True, stop=True,
                    )
                for idx, jt in enumerate(g):
                    if KT * jt >= q0:
                        trim = KT * jt - q0
                        nc.vector.tensor_add(
                            sc[:, BQ * idx + trim : BQ * idx + trim + 128],
                            sc[:, BQ * idx + trim : BQ * idx + trim + 128],
                            lu_s[:],
                        )
                ex = sb.tile([128, G * BQ], BF16, tag="expT", name="expT", bufs=3)
                nc.scalar.activation(
                    ex[:, 0 : BQ * len(g)], sc[:],
                    mybir.ActivationFunctionType.Exp, scale=0.125,
                )
                if pending is not None:
                    emit_pv(*pending)
                pending = (h, g, ex)
            emit_pv(*pending)

            for h in range(2):
                hb = 64 * h
                den_s = sb.tile([1, BQ], F32, tag=f"dens{h}", name=f"dens{h}", bufs=2)
                den_f = sb.tile([1, BQ], F32, tag=f"denf{h}", name=f"denf{h}", bufs=2)
                den_b = sb.tile([1, BQ], BF16, tag=f"denb{h}", name=f"denb{h}", bufs=2)
                nc.vector.tensor_copy(den_s[:], outT[h][64:65, :])
                nc.vector.tensor_copy(attnT[hb : hb + 64, q0 : q0 + BQ], outT[h][0:64, :])
                nc.vector.reciprocal_approx_fast(den_f[:], den_s[:])
                nc.vector.tensor_copy(den_b[:], den_f[:])
                bc = psc.tile([64, BQ], F32, tag="sc", name="bc")
                nc.tensor.matmul(bc[:], ones_c[:], den_b[:], start=True, stop=True)
                nc.vector.tensor_mul(
                    attnT[hb : hb + 64, q0 : q0 + BQ],
                    attnT[hb : hb + 64, q0 : q0 + BQ],
                    bc[:],
                )
            nc.sync.dma_start(
                a2a_in[128 * Q : 128 * (Q + 1), :], attnT[:, q0 : q0 + BQ]
            )

        for nb in range(NQB):
            proj_block(nb)
            if nb >= 1:
                attn_block(nb - 1)
        attn_block(NQB - 1)

        nc.gpsimd.collective_compute(
            "AllToAll",
            mybir.AluOpType.bypass,
            replica_groups=[list(range(N_CORES))],
            ins=[a2a_in.ap().opt()],
            outs=[a2a_out.ap().opt()],
        )
        for t in range(NFT):
            nc.sync.dma_start(aT[t][:], a2a_out[128 * t : 128 * (t + 1), :])

        for it in range(CHUNK // 128):
            for oh in range(D // 512):
                p = psc.tile([128, 512], F32, tag="sc", name="p_o")
                for t in range(NFT):
                    nc.tensor.matmul(
                        p[:],
                        aT[t][:, 128 * it : 128 * (it + 1)],
                        wo_s[t][:, 512 * oh : 512 * (oh + 1)],
                        start=(t == 0), stop=(t == NFT - 1),
                    )
                ot = sb.tile([128, 512], F32, tag="oflush", name="ot", bufs=2)
                nc.scalar.copy(ot[:], p[:])
                nc.sync.dma_start(
                    out[128 * it : 128 * (it + 1), 512 * oh : 512 * (oh + 1)], ot[:]
                )

    nc.compile()
    return nc


def _host_prep(x, Wq, Wk, Wv, Wo):
    bf = ml_dtypes.bfloat16
    perm = np.empty(HD, dtype=np.int64)
    perm[:32] = np.arange(0, 64, 2)
    perm[32:] = np.arange(1, 64, 2)

    inv_freq = 1.0 / (10000.0 ** (np.arange(0, HD, 2, dtype=np.float32) / HD))
    fr = np.outer(np.arange(S, dtype=np.float32), inv_freq)
    cosA = np.cos(fr).T
    sinA = np.sin(fr).T
    cosP = np.tile(np.concatenate([cosA, cosA], 0), (2, 1)).astype(bf)
    sinN = np.tile(np.concatenate([-sinA, sinA], 0), (2, 1)).astype(bf)
    lu = np.tril(np.full((128, 128), -400.0, np.float32), k=-1).astype(bf)

    xT = np.ascontiguousarray(x.reshape(S, D).T).astype(bf)
    woT = np.ascontiguousarray(np.asarray(Wo, np.float32).T).astype(bf)

    in_maps = []
    for c in range(N_CORES):
        rows = np.concatenate([128 * c + 64 * h + perm for h in range(2)])
        in_maps.append(
            {
                "xT": xT,
                "wq": np.ascontiguousarray(np.asarray(Wq, np.float32)[rows].T).astype(bf),
                "wk": np.ascontiguousarray(np.asarray(Wk, np.float32)[rows].T).astype(bf),
                "wv": np.ascontiguousarray(
                    np.asarray(Wv, np.float32)[128 * c : 128 * (c + 1)].T
                ).astype(bf),
                "wo": woT,
                "cosP": cosP,
                "sinN": sinN,
                "lu": lu,
            }
        )
    return in_maps


_NC_CACHE = None


def kernel(x, Wq, Wk, Wv, Wo):
    global _NC_CACHE
    if _NC_CACHE is None:
        _NC_CACHE = _build()
    nc = _NC_CACHE
    in_maps = _host_prep(
        np.asarray(x, np.float32),
        np.asarray(Wq, np.float32),
        np.asarray(Wk, np.float32),
        np.asarray(Wv, np.float32),
        np.asarray(Wo, np.float32),
    )
    res = run_bass_kernel_spmd(nc, in_maps, core_ids=list(range(N_CORES)))
    full = np.concatenate([res.results[c]["out"] for c in range(N_CORES)], axis=0)
    return full.reshape(1, S, D).astype(np.float32)


# revision 12
# speedup vs baseline: 1.3107x; 1.1407x over previous
"""nn_MultiHeadAttention TRN2 kernel: 8-core tensor-parallel (2 heads/core).

Self-contained: builds and compiles the Bass/Tile SPMD program on first call,
shards the full inputs per-core on the host, runs via run_bass_kernel_spmd,
and concatenates the per-core sequence-block outputs into the full output.

v3 design (per core, 2 heads of 16, head_dim 64, S=4096, D=1024):
  - feature-major xT [D,S]; q/k projected with RoPE-permuted transposed
    weight shards laid out so the rotary partner row sits in the same
    32-partition quadrant: rotation = one DVE stream_shuffle (16-row swap)
    instead of SBUF-SBUF DMAs.
  - v seq-major into a 4-D v_aug tile [128, 32, 2, 65] whose 65th column is
    1.0 (softmax denominator produced by the PV matmul directly).
  - projection and attention emission interleaved per 512-block so the PE
    pipeline never drains (DVFS: sustained activity ramps PE 1.2->2.4 GHz).
  - flash attention on transposed score tiles: per Q-block the heads'
    exp-groups alternate and PV emission is skewed one group behind scores,
    keeping TensorE busy while ScalarE computes exp.
  - normalization fully decoupled from the PE: outT rows copied to SBUF,
    reciprocal_approx_fast + gpsimd partition_broadcast + in-place DVE
    multiply; only the A2A staging DMA waits on it.
  - per-Q-block staging DMAs feed one AllToAll (head-split -> seq-split);
    final projection against full Wo.T; each core emits out[512, 1024] f32.
"""

from contextlib import ExitStack

import numpy as np
import ml_dtypes

import concourse.tile as tile
from concourse import bacc, mybir
from concourse.bass_utils import run_bass_kernel_spmd

F32 = mybir.dt.float32
BF16 = mybir.dt.bfloat16

S = 4096
D = 1024
HD = 64
N_CORES = 8
KT = 128
BQ = 512
CHUNK = S // N_CORES
NFT = D // 128
NKT = S // KT
NQB = S // BQ
G = 3

# stream_shuffle mask: swap 16-row halves within each 32-partition quadrant
SWAP16 = [16 + i for i in range(16)] + list(range(16))


def _build():
    nc = bacc.Bacc("TRN2", target_bir_lowering=False, debug=False, num_devices=N_CORES)

    xT = nc.dram_tensor("xT", [D, S], BF16, kind="ExternalInput")
    wq = nc.dram_tensor("wq", [D, 128], BF16, kind="ExternalInput")
    wk = nc.dram_tensor("wk", [D, 128], BF16, kind="ExternalInput")
    wv = nc.dram_tensor("wv", [D, 128], BF16, kind="ExternalInput")
    wo = nc.dram_tensor("wo", [D, D], BF16, kind="ExternalInput")
    cosP = nc.dram_tensor("cosP", [128, S], BF16, kind="ExternalInput")
    sinN = nc.dram_tensor("sinN", [128, S], BF16, kind="ExternalInput")
    lu = nc.dram_tensor("lu", [128, 128], BF16, kind="ExternalInput")
    out = nc.dram_tensor("out", [CHUNK, D], F32, kind="ExternalOutput")

    a2a_in = nc.dram_tensor("a2a_in", [N_CORES * 128, CHUNK], BF16)
    a2a_out = nc.dram_tensor("a2a_out", [N_CORES * 128, CHUNK], BF16)

    with tile.TileContext(nc) as tc, ExitStack() as ctx:
        sb = ctx.enter_context(tc.tile_pool(name="sb", bufs=1))
        xt_s = [sb.tile([128, S], BF16, tag=f"xt{t}", name=f"xt{t}") for t in range(NFT)]
        wq_s = sb.tile([128, NFT * 128], BF16, tag="wq", name="wq_s")
        wk_s = sb.tile([128, NFT * 128], BF16, tag="wk", name="wk_s")
        wv_s = sb.tile([128, NFT * 128], BF16, tag="wv", name="wv_s")
        wo_s = sb.tile([128, NFT, D], BF16, tag="wo", name="wo_s")
        cos_s = sb.tile([128, S], BF16, tag="cos", name="cos_s")
        sin_s = sb.tile([128, S], BF16, tag="sin", name="sin_s")
        lu_s = sb.tile([128, 128], BF16, tag="lu", name="lu_s")
        qA = sb.tile([128, S], BF16, tag="qA", name="qA")
        kA = sb.tile([128, S], BF16, tag="kA", name="kA")
        qT = sb.tile([128, S], BF16, tag="qT", name="qT")
        kT_ = sb.tile([128, S], BF16, tag="kT", name="kT_")
        v_aug = sb.tile([128, NKT, 2, 65], BF16, tag="vaug", name="v_aug")
        attnT = sb.tile([128, S], BF16, tag="attnT", name="attnT")
        aT = [sb.tile([128, CHUNK], BF16, tag=f"aT{t}", name=f"aT{t}") for t in range(NFT)]

        nc.vector.memset(v_aug[:], 1.0)

        nc.sync.dma_start(
            wq_s[:].rearrange("p (t c) -> p t c", t=NFT),
            wq[:, :].rearrange("(t p) c -> p t c", t=NFT),
        )
        nc.sync.dma_start(
            wk_s[:].rearrange("p (t c) -> p t c", t=NFT),
            wk[:, :].rearrange("(t p) c -> p t c", t=NFT),
        )
        nc.sync.dma_start(
            wv_s[:].rearrange("p (t c) -> p t c", t=NFT),
            wv[:, :].rearrange("(t p) c -> p t c", t=NFT),
        )
        nc.sync.dma_start(cos_s[:], cosP[:, :])
        nc.sync.dma_start(sin_s[:], sinN[:, :])
        nc.sync.dma_start(lu_s[:], lu[:, :])
        nc.sync.dma_start(
            wo_s[:], wo[:, :].rearrange("(t p) c -> p t c", t=NFT)
        )
        # x loads issued from the (otherwise idle) scalar queue, 1KB rows
        for nb2 in range(NQB // 2):
            c = slice(1024 * nb2, 1024 * (nb2 + 1))
            for t in range(NFT):
                nc.scalar.dma_start(xt_s[t][:, c], xT[128 * t : 128 * (t + 1), c])

        psc = ctx.enter_context(tc.tile_pool(name="psc", bufs=2, space="PSUM"))
        ppv = ctx.enter_context(tc.tile_pool(name="ppv", bufs=2, space="PSUM"))

        def proj_block(nb):
            c = slice(BQ * nb, BQ * (nb + 1))
            qp = psc.tile([128, BQ], F32, tag="sc", name="qp")
            for t in range(NFT):
                nc.tensor.matmul(
                    qp[:], wq_s[:, 128 * t : 128 * (t + 1)], xt_s[t][:, c],
                    start=(t == 0), stop=(t == NFT - 1),
                )
            kp = psc.tile([128, BQ], F32, tag="sc", name="kp")
            for t in range(NFT):
                nc.tensor.matmul(
                    kp[:], wk_s[:, 128 * t : 128 * (t + 1)], xt_s[t][:, c],
                    start=(t == 0), stop=(t == NFT - 1),
                )
            nc.vector.tensor_copy(qA[:, c], qp[:])
            nc.vector.tensor_copy(kA[:, c], kp[:])
            vp = psc.tile([128, BQ], F32, tag="sc", name="vp")
            for u in range(4):
                st = slice(BQ * nb + 128 * u, BQ * nb + 128 * (u + 1))
                for t in range(NFT):
                    nc.tensor.matmul(
                        vp[:, 128 * u : 128 * (u + 1)], xt_s[t][:, st],
                        wv_s[:, 128 * t : 128 * (t + 1)],
                        start=(t == 0), stop=(t == NFT - 1),
                    )
            nc.vector.tensor_copy(
                v_aug[:, 4 * nb : 4 * (nb + 1), :, 0:64],
                vp[:].rearrange("p (u h c) -> p u h c", u=4, h=2, c=64),
            )
            qBt = sb.tile([128, BQ], BF16, tag="qB", name="qBt", bufs=2)
            kBt = sb.tile([128, BQ], BF16, tag="kB", name="kBt", bufs=2)
            nc.vector.stream_shuffle(qBt[:], qA[:, c], SWAP16)
            nc.vector.stream_shuffle(kBt[:], kA[:, c], SWAP16)
            for A, B, R in ((qA, qBt, qT), (kA, kBt, kT_)):
                nc.vector.tensor_mul(R[:, c], A[:, c], cos_s[:, c])
                nc.vector.tensor_mul(B[:], B[:], sin_s[:, c])
                nc.vector.tensor_add(R[:, c], R[:, c], B[:])

        def attn_block(Q):
            q0 = BQ * Q
            n_jt = 4 * (Q + 1)
            jts = list(range(n_jt))
            groups = [jts[i : i + G] for i in range(0, n_jt, G)]
            seq = [(h, g) for g in groups for h in range(2)]
            outT = {}

            def emit_pv(h, g, ex):
                if g[0] == 0:
                    outT[h] = ppv.tile([65, BQ], F32, tag="pv", name=f"outT{h}")
                for idx, jt in enumerate(g):
                    trim = max(0, KT * jt - q0)
                    nc.tensor.matmul(
                        outT[h][:, trim:BQ],
                        v_aug[:, jt, h, :],
                        ex[:, BQ * idx + trim : BQ * (idx + 1)],
                        start=(jt == 0),
                        stop=(jt == n_jt - 1),
                    )

            pending = None
            for h, g in seq:
                hb = 64 * h
                sc = psc.tile([128, BQ * len(g)], F32, tag="sc", name="sc")
                for idx, jt in enumerate(g):
                    nc.tensor.matmul(
                        sc[:, BQ * idx : BQ * (idx + 1)],
                        kT_[hb : hb + 64, KT * jt : KT * (jt + 1)],
                        qT[hb : hb + 64, q0 : q0 + BQ],
                        start=True, stop=True,
                    )
                for idx, jt in enumerate(g):
                    if KT * jt >= q0:
                        trim = KT * jt - q0
                        nc.vector.tensor_add(
                            sc[:, BQ * idx + trim : BQ * idx + trim + 128],
                            sc[:, BQ * idx + trim : BQ * idx + trim + 128],
                            lu_s[:],
                        )
                ex = sb.tile([128, G * BQ], BF16, tag="expT", name="expT", bufs=3)
                nc.scalar.activation(
                    ex[:, 0 : BQ * len(g)], sc[:],
                    mybir.ActivationFunctionType.Exp, scale=0.125,
                )
                if pending is not None:
                    emit_pv(*pending)
                pending = (h, g, ex)
            emit_pv(*pending)

            den_s = sb.tile([1, 2 * BQ], F32, tag="dens", name="dens", bufs=2)
            den_f = sb.tile([1, 2 * BQ], F32, tag="denf", name="denf", bufs=2)
            bc_sb = sb.tile([128, 2 * BQ], F32, tag="bc", name="bc", bufs=2)
            for h in range(2):
                hb = 64 * h
                nc.vector.tensor_copy(den_s[:, BQ * h : BQ * (h + 1)], outT[h][64:65, :])
                nc.vector.tensor_copy(attnT[hb : hb + 64, q0 : q0 + BQ], outT[h][0:64, :])
            nc.vector.reciprocal_approx_fast(den_f[:], den_s[:])
            nc.gpsimd.partition_broadcast(bc_sb[:], den_f[:])
            for h in range(2):
                hb = 64 * h
                nc.vector.tensor_mul(
                    attnT[hb : hb + 64, q0 : q0 + BQ],
                    attnT[hb : hb + 64, q0 : q0 + BQ],
                    bc_sb[hb : hb + 64, BQ * h : BQ * (h + 1)],
                )
            nc.sync.dma_start(
                a2a_in[128 * Q : 128 * (Q + 1), :], attnT[:, q0 : q0 + BQ]
            )

        for nb in range(NQB):
            proj_block(nb)
            if nb >= 1:
                attn_block(nb - 1)
        attn_block(NQB - 1)

        nc.gpsimd.collective_compute(
            "AllToAll",
            mybir.AluOpType.bypass,
            replica_groups=[list(range(N_CORES))],
            ins=[a2a_in.ap().opt()],
            outs=[a2a_out.ap().opt()],
        )
        for t in range(NFT):
            nc.sync.dma_start(aT[t][:], a2a_out[128 * t : 128 * (t + 1), :])

        for it in range(CHUNK // 128):
            for oh in range(D // 512):
                p = psc.tile([128, 512], F32, tag="sc", name="p_o")
                for t in range(NFT):
                    nc.tensor.matmul(
                        p[:],
                        aT[t][:, 128 * it : 128 * (it + 1)],
                        wo_s[:, t, 512 * oh : 512 * (oh + 1)],
                        start=(t == 0), stop=(t == NFT - 1),
                    )
                ot = sb.tile([128, 512], F32, tag="oflush", name="ot", bufs=2)
                nc.scalar.copy(ot[:], p[:])
                nc.sync.dma_start(
                    out[128 * it : 128 * (it + 1), 512 * oh : 512 * (oh + 1)], ot[:]
                )

    nc.compile()
    return nc


def _host_prep(x, Wq, Wk, Wv, Wo):
    bf = ml_dtypes.bfloat16
    # quadrant-local RoPE layout: within each head (64 rows = 2 quadrants of
    # 32), quadrant q holds pairs [16q:16q+16): rows 32q+j = dim 2(16q+j)
    # (even), rows 32q+16+j = dim 2(16q+j)+1 (odd). The rotary partner of a
    # row is then 16 rows away inside the same quadrant (stream_shuffle).
    perm = np.empty(HD, dtype=np.int64)
    pair = np.empty(HD, dtype=np.int64)
    sign = np.empty(HD, dtype=np.float32)
    for q in range(2):
        for j in range(16):
            perm[32 * q + j] = 2 * (16 * q + j)
            perm[32 * q + 16 + j] = 2 * (16 * q + j) + 1
            pair[32 * q + j] = 16 * q + j
            pair[32 * q + 16 + j] = 16 * q + j
            sign[32 * q + j] = -1.0
            sign[32 * q + 16 + j] = 1.0

    inv_freq = 1.0 / (10000.0 ** (np.arange(0, HD, 2, dtype=np.float32) / HD))
    fr = np.outer(np.arange(S, dtype=np.float32), inv_freq)
    cosA = np.cos(fr).T  # [32, S] per pair index
    sinA = np.sin(fr).T
    cosH = cosA[pair]                      # [64, S]
    sinH = sinA[pair] * sign[:, None]      # [64, S]
    cosP = np.tile(cosH, (2, 1)).astype(bf)
    sinN = np.tile(sinH, (2, 1)).astype(bf)
    lu = np.tril(np.full((128, 128), -400.0, np.float32), k=-1).astype(bf)

    xT = np.ascontiguousarray(x.reshape(S, D).T).astype(bf)
    woT = np.ascontiguousarray(np.asarray(Wo, np.float32).T).astype(bf)

    in_maps = []
    for c in range(N_CORES):
        rows = np.concatenate([128 * c + 64 * h + perm for h in range(2)])
        in_maps.append(
            {
                "xT": xT,
                "wq": np.ascontiguousarray(np.asarray(Wq, np.float32)[rows].T).astype(bf),
                "wk": np.ascontiguousarray(np.asarray(Wk, np.float32)[rows].T).astype(bf),
                "wv": np.ascontiguousarray(
                    np.asarray(Wv, np.float32)[128 * c : 128 * (c + 1)].T
                ).astype(bf),
                "wo": woT,
                "cosP": cosP,
                "sinN": sinN,
                "lu": lu,
            }
        )
    return in_maps


_NC_CACHE = None


def kernel(x, Wq, Wk, Wv, Wo):
    global _NC_CACHE
    if _NC_CACHE is None:
        _NC_CACHE = _build()
    nc = _NC_CACHE
    in_maps = _host_prep(
        np.asarray(x, np.float32),
        np.asarray(Wq, np.float32),
        np.asarray(Wk, np.float32),
        np.asarray(Wv, np.float32),
        np.asarray(Wo, np.float32),
    )
    res = run_bass_kernel_spmd(nc, in_maps, core_ids=list(range(N_CORES)))
    full = np.concatenate([res.results[c]["out"] for c in range(N_CORES)], axis=0)
    return full.reshape(1, S, D).astype(np.float32)


# revision 13
# speedup vs baseline: 1.6460x; 1.2558x over previous
"""nn_MultiHeadAttention TRN2 kernel: 8-core tensor-parallel (2 heads/core).

Self-contained: builds and compiles the Bass/Tile SPMD program on first call,
shards the full inputs per-core on the host, runs via run_bass_kernel_spmd,
and concatenates the per-core sequence-block outputs into the full output.

v3 design (per core, 2 heads of 16, head_dim 64, S=4096, D=1024):
  - feature-major xT [D,S]; q/k projected with RoPE-permuted transposed
    weight shards laid out so the rotary partner row sits in the same
    32-partition quadrant: rotation = one DVE stream_shuffle (16-row swap)
    instead of SBUF-SBUF DMAs.
  - v seq-major into a 4-D v_aug tile [128, 32, 2, 65] whose 65th column is
    1.0 (softmax denominator produced by the PV matmul directly).
  - projection and attention emission interleaved per 512-block so the PE
    pipeline never drains (DVFS: sustained activity ramps PE 1.2->2.4 GHz).
  - flash attention on transposed score tiles: per Q-block the heads'
    exp-groups alternate and PV emission is skewed one group behind scores,
    keeping TensorE busy while ScalarE computes exp.
  - normalization fully decoupled from the PE: outT rows copied to SBUF,
    reciprocal_approx_fast + gpsimd partition_broadcast + in-place DVE
    multiply; only the A2A staging DMA waits on it.
  - per-Q-block staging DMAs feed one AllToAll (head-split -> seq-split);
    final projection against full Wo.T; each core emits out[512, 1024] f32.
"""

from contextlib import ExitStack

import numpy as np
import ml_dtypes

import concourse.tile as tile
from concourse import bacc, mybir
from concourse.bass_utils import run_bass_kernel_spmd

F32 = mybir.dt.float32
BF16 = mybir.dt.bfloat16

S = 4096
D = 1024
HD = 64
N_CORES = 8
KT = 128
BQ = 512
CHUNK = S // N_CORES
NFT = D // 128
NKT = S // KT
NQB = S // BQ
G = 2

# stream_shuffle mask: swap 16-row halves within each 32-partition quadrant
SWAP16 = [16 + i for i in range(16)] + list(range(16))


def _build():
    nc = bacc.Bacc("TRN2", target_bir_lowering=False, debug=False, num_devices=N_CORES)

    xT = nc.dram_tensor("xT", [D, S], BF16, kind="ExternalInput")
    wq = nc.dram_tensor("wq", [D, 128], BF16, kind="ExternalInput")
    wk = nc.dram_tensor("wk", [D, 128], BF16, kind="ExternalInput")
    wv = nc.dram_tensor("wv", [D, 128], BF16, kind="ExternalInput")
    wo = nc.dram_tensor("wo", [D, D], BF16, kind="ExternalInput")
    cosP = nc.dram_tensor("cosP", [128, S], BF16, kind="ExternalInput")
    sinN = nc.dram_tensor("sinN", [128, S], BF16, kind="ExternalInput")
    lu = nc.dram_tensor("lu", [128, 128], BF16, kind="ExternalInput")
    out = nc.dram_tensor("out", [CHUNK, D], F32, kind="ExternalOutput")

    a2a_in = nc.dram_tensor("a2a_in", [N_CORES * 128, CHUNK], BF16)
    a2a_out = nc.dram_tensor("a2a_out", [N_CORES * 128, CHUNK], BF16)

    with tile.TileContext(nc) as tc, ExitStack() as ctx:
        sb = ctx.enter_context(tc.tile_pool(name="sb", bufs=1))
        xt_s = [sb.tile([128, S], BF16, tag=f"xt{t}", name=f"xt{t}") for t in range(NFT)]
        wq_s = sb.tile([128, NFT * 128], BF16, tag="wq", name="wq_s")
        wk_s = sb.tile([128, NFT * 128], BF16, tag="wk", name="wk_s")
        wv_s = sb.tile([128, NFT * 128], BF16, tag="wv", name="wv_s")
        wo_s = sb.tile([128, NFT, D], BF16, tag="wo", name="wo_s")
        cos_s = sb.tile([128, S], BF16, tag="cos", name="cos_s")
        sin_s = sb.tile([128, S], BF16, tag="sin", name="sin_s")
        lu_s = sb.tile([128, 128], BF16, tag="lu", name="lu_s")
        qA = sb.tile([128, S], BF16, tag="qA", name="qA")
        kA = sb.tile([128, S], BF16, tag="kA", name="kA")
        qT = sb.tile([128, S], BF16, tag="qT", name="qT")
        kT_ = sb.tile([128, S], BF16, tag="kT", name="kT_")
        v_aug = sb.tile([128, NKT, 2, 65], BF16, tag="vaug", name="v_aug")
        attnT = sb.tile([128, S], BF16, tag="attnT", name="attnT")
        aT = [sb.tile([128, CHUNK], BF16, tag=f"aT{t}", name=f"aT{t}") for t in range(NFT)]

        nc.vector.memset(v_aug[:], 1.0)

        nc.sync.dma_start(
            wq_s[:].rearrange("p (t c) -> p t c", t=NFT),
            wq[:, :].rearrange("(t p) c -> p t c", t=NFT),
        )
        nc.sync.dma_start(
            wk_s[:].rearrange("p (t c) -> p t c", t=NFT),
            wk[:, :].rearrange("(t p) c -> p t c", t=NFT),
        )
        nc.sync.dma_start(
            wv_s[:].rearrange("p (t c) -> p t c", t=NFT),
            wv[:, :].rearrange("(t p) c -> p t c", t=NFT),
        )
        nc.sync.dma_start(cos_s[:], cosP[:, :])
        nc.sync.dma_start(sin_s[:], sinN[:, :])
        nc.sync.dma_start(lu_s[:], lu[:, :])
        nc.sync.dma_start(
            wo_s[:], wo[:, :].rearrange("(t p) c -> p t c", t=NFT)
        )
        # x loads issued from the (otherwise idle) scalar queue, 1KB rows
        for nb2 in range(NQB // 2):
            c = slice(1024 * nb2, 1024 * (nb2 + 1))
            for t in range(NFT):
                nc.gpsimd.dma_start(xt_s[t][:, c], xT[128 * t : 128 * (t + 1), c])

        psc = ctx.enter_context(tc.tile_pool(name="psc", bufs=3, space="PSUM"))
        ppv = ctx.enter_context(tc.tile_pool(name="ppv", bufs=2, space="PSUM"))

        def proj_block(nb):
            c = slice(BQ * nb, BQ * (nb + 1))
            qp = psc.tile([128, BQ], F32, tag="sc", name="qp")
            for t in range(NFT):
                nc.tensor.matmul(
                    qp[:], wq_s[:, 128 * t : 128 * (t + 1)], xt_s[t][:, c],
                    start=(t == 0), stop=(t == NFT - 1),
                )
            kp = psc.tile([128, BQ], F32, tag="sc", name="kp")
            for t in range(NFT):
                nc.tensor.matmul(
                    kp[:], wk_s[:, 128 * t : 128 * (t + 1)], xt_s[t][:, c],
                    start=(t == 0), stop=(t == NFT - 1),
                )
            nc.vector.tensor_copy(qA[:, c], qp[:])
            nc.vector.tensor_copy(kA[:, c], kp[:])
            vp = psc.tile([128, BQ], F32, tag="sc", name="vp")
            for u in range(4):
                st = slice(BQ * nb + 128 * u, BQ * nb + 128 * (u + 1))
                for t in range(NFT):
                    nc.tensor.matmul(
                        vp[:, 128 * u : 128 * (u + 1)], xt_s[t][:, st],
                        wv_s[:, 128 * t : 128 * (t + 1)],
                        start=(t == 0), stop=(t == NFT - 1),
                    )
            nc.vector.tensor_copy(
                v_aug[:, 4 * nb : 4 * (nb + 1), :, 0:64],
                vp[:].rearrange("p (u h c) -> p u h c", u=4, h=2, c=64),
            )
            qBt = sb.tile([128, BQ], BF16, tag="qB", name="qBt", bufs=2)
            kBt = sb.tile([128, BQ], BF16, tag="kB", name="kBt", bufs=2)
            nc.vector.stream_shuffle(qBt[:], qA[:, c], SWAP16)
            nc.vector.stream_shuffle(kBt[:], kA[:, c], SWAP16)
            for A, B, R in ((qA, qBt, qT), (kA, kBt, kT_)):
                nc.vector.tensor_mul(R[:, c], A[:, c], cos_s[:, c])
                nc.vector.tensor_mul(B[:], B[:], sin_s[:, c])
                nc.vector.tensor_add(R[:, c], R[:, c], B[:])

        def attn_block(Q):
            q0 = BQ * Q
            n_jt = 4 * (Q + 1)
            jts = list(range(n_jt))
            groups = [jts[i : i + G] for i in range(0, n_jt, G)]
            seq = [(h, g) for g in groups for h in range(2)]
            outT = {}

            def emit_pv(h, g, ex):
                if g[0] == 0:
                    outT[h] = ppv.tile([65, BQ], F32, tag="pv", name=f"outT{h}")
                for idx, jt in enumerate(g):
                    trim = max(0, KT * jt - q0)
                    nc.tensor.matmul(
                        outT[h][:, trim:BQ],
                        v_aug[:, jt, h, :],
                        ex[:, BQ * idx + trim : BQ * (idx + 1)],
                        start=(jt == 0),
                        stop=(jt == n_jt - 1),
                    )

            pending = None
            for h, g in seq:
                hb = 64 * h
                sc = psc.tile([128, BQ * len(g)], F32, tag="sc", name="sc")
                for idx, jt in enumerate(g):
                    nc.tensor.matmul(
                        sc[:, BQ * idx : BQ * (idx + 1)],
                        kT_[hb : hb + 64, KT * jt : KT * (jt + 1)],
                        qT[hb : hb + 64, q0 : q0 + BQ],
                        start=True, stop=True,
                    )
                for idx, jt in enumerate(g):
                    if KT * jt >= q0:
                        trim = KT * jt - q0
                        nc.vector.tensor_add(
                            sc[:, BQ * idx + trim : BQ * idx + trim + 128],
                            sc[:, BQ * idx + trim : BQ * idx + trim + 128],
                            lu_s[:],
                        )
                ex = sb.tile([128, G * BQ], BF16, tag="expT", name="expT", bufs=3)
                nc.scalar.activation(
                    ex[:, 0 : BQ * len(g)], sc[:],
                    mybir.ActivationFunctionType.Exp, scale=0.125,
                )
                if pending is not None:
                    emit_pv(*pending)
                pending = (h, g, ex)
            emit_pv(*pending)

            den_s = sb.tile([1, 2 * BQ], F32, tag="dens", name="dens", bufs=2)
            den_f = sb.tile([1, 2 * BQ], F32, tag="denf", name="denf", bufs=2)
            bc_sb = sb.tile([128, 2 * BQ], F32, tag="bc", name="bc", bufs=2)
            for h in range(2):
                hb = 64 * h
                nc.vector.tensor_copy(den_s[:, BQ * h : BQ * (h + 1)], outT[h][64:65, :])
                nc.vector.tensor_copy(attnT[hb : hb + 64, q0 : q0 + BQ], outT[h][0:64, :])
            nc.vector.reciprocal_approx_fast(den_f[:], den_s[:])
            nc.gpsimd.partition_broadcast(bc_sb[:], den_f[:])
            for h in range(2):
                hb = 64 * h
                nc.vector.tensor_mul(
                    attnT[hb : hb + 64, q0 : q0 + BQ],
                    attnT[hb : hb + 64, q0 : q0 + BQ],
                    bc_sb[hb : hb + 64, BQ * h : BQ * (h + 1)],
                )
            nc.sync.dma_start(
                a2a_in[128 * Q : 128 * (Q + 1), :], attnT[:, q0 : q0 + BQ]
            )

        for nb in range(NQB):
            proj_block(nb)
            if nb >= 1:
                attn_block(nb - 1)
        attn_block(NQB - 1)

        nc.gpsimd.collective_compute(
            "AllToAll",
            mybir.AluOpType.bypass,
            replica_groups=[list(range(N_CORES))],
            ins=[a2a_in.ap().opt()],
            outs=[a2a_out.ap().opt()],
        )
        for t in range(NFT):
            nc.sync.dma_start(aT[t][:], a2a_out[128 * t : 128 * (t + 1), :])

        for it in range(CHUNK // 128):
            for oh in range(D // 512):
                p = psc.tile([128, 512], F32, tag="sc", name="p_o")
                for t in range(NFT):
                    nc.tensor.matmul(
                        p[:],
                        aT[t][:, 128 * it : 128 * (it + 1)],
                        wo_s[:, t, 512 * oh : 512 * (oh + 1)],
                        start=(t == 0), stop=(t == NFT - 1),
                    )
                ot = sb.tile([128, 512], F32, tag="oflush", name="ot", bufs=2)
                nc.scalar.copy(ot[:], p[:])
                nc.sync.dma_start(
                    out[128 * it : 128 * (it + 1), 512 * oh : 512 * (oh + 1)], ot[:]
                )

    nc.compile()
    return nc


def _host_prep(x, Wq, Wk, Wv, Wo):
    bf = ml_dtypes.bfloat16
    # quadrant-local RoPE layout: within each head (64 rows = 2 quadrants of
    # 32), quadrant q holds pairs [16q:16q+16): rows 32q+j = dim 2(16q+j)
    # (even), rows 32q+16+j = dim 2(16q+j)+1 (odd). The rotary partner of a
    # row is then 16 rows away inside the same quadrant (stream_shuffle).
    perm = np.empty(HD, dtype=np.int64)
    pair = np.empty(HD, dtype=np.int64)
    sign = np.empty(HD, dtype=np.float32)
    for q in range(2):
        for j in range(16):
            perm[32 * q + j] = 2 * (16 * q + j)
            perm[32 * q + 16 + j] = 2 * (16 * q + j) + 1
            pair[32 * q + j] = 16 * q + j
            pair[32 * q + 16 + j] = 16 * q + j
            sign[32 * q + j] = -1.0
            sign[32 * q + 16 + j] = 1.0

    inv_freq = 1.0 / (10000.0 ** (np.arange(0, HD, 2, dtype=np.float32) / HD))
    fr = np.outer(np.arange(S, dtype=np.float32), inv_freq)
    cosA = np.cos(fr).T  # [32, S] per pair index
    sinA = np.sin(fr).T
    cosH = cosA[pair]                      # [64, S]
    sinH = sinA[pair] * sign[:, None]      # [64, S]
    cosP = np.tile(cosH, (2, 1)).astype(bf)
    sinN = np.tile(sinH, (2, 1)).astype(bf)
    lu = np.tril(np.full((128, 128), -400.0, np.float32), k=-1).astype(bf)

    xT = np.ascontiguousarray(x.reshape(S, D).T).astype(bf)
    woT = np.ascontiguousarray(np.asarray(Wo, np.float32).T).astype(bf)

    in_maps = []
    for c in range(N_CORES):
        rows = np.concatenate([128 * c + 64 * h + perm for h in range(2)])
        in_maps.append(
            {
                "xT": xT,
                "wq": np.ascontiguousarray(np.asarray(Wq, np.float32)[rows].T).astype(bf),
                "wk": np.ascontiguousarray(np.asarray(Wk, np.float32)[rows].T).astype(bf),
                "wv": np.ascontiguousarray(
                    np.asarray(Wv, np.float32)[128 * c : 128 * (c + 1)].T
                ).astype(bf),
                "wo": woT,
                "cosP": cosP,
                "sinN": sinN,
                "lu": lu,
            }
        )
    return in_maps


_NC_CACHE = None


def kernel(x, Wq, Wk, Wv, Wo):
    global _NC_CACHE
    if _NC_CACHE is None:
        _NC_CACHE = _build()
    nc = _NC_CACHE
    in_maps = _host_prep(
        np.asarray(x, np.float32),
        np.asarray(Wq, np.float32),
        np.asarray(Wk, np.float32),
        np.asarray(Wv, np.float32),
        np.asarray(Wo, np.float32),
    )
    res = run_bass_kernel_spmd(nc, in_maps, core_ids=list(range(N_CORES)))
    full = np.concatenate([res.results[c]["out"] for c in range(N_CORES)], axis=0)
    return full.reshape(1, S, D).astype(np.float32)


# revision 14
# speedup vs baseline: 1.6502x; 1.0025x over previous
"""nn_MultiHeadAttention TRN2 kernel: 8-core tensor-parallel (2 heads/core).

Self-contained: builds and compiles the Bass/Tile SPMD program on first call,
shards the full inputs per-core on the host, runs via run_bass_kernel_spmd,
and concatenates the per-core sequence-block outputs into the full output.

v3 design (per core, 2 heads of 16, head_dim 64, S=4096, D=1024):
  - feature-major xT [D,S]; q/k projected with RoPE-permuted transposed
    weight shards laid out so the rotary partner row sits in the same
    32-partition quadrant: rotation = one DVE stream_shuffle (16-row swap)
    instead of SBUF-SBUF DMAs.
  - v seq-major into a 4-D v_aug tile [128, 32, 2, 65] whose 65th column is
    1.0 (softmax denominator produced by the PV matmul directly).
  - projection and attention emission interleaved per 512-block so the PE
    pipeline never drains (DVFS: sustained activity ramps PE 1.2->2.4 GHz).
  - flash attention on transposed score tiles: per Q-block the heads'
    exp-groups alternate and PV emission is skewed one group behind scores,
    keeping TensorE busy while ScalarE computes exp.
  - normalization fully decoupled from the PE: outT rows copied to SBUF,
    reciprocal_approx_fast + gpsimd partition_broadcast + in-place DVE
    multiply; only the A2A staging DMA waits on it.
  - per-Q-block staging DMAs feed one AllToAll (head-split -> seq-split);
    final projection against full Wo.T; each core emits out[512, 1024] f32.
"""

from contextlib import ExitStack

import numpy as np
import ml_dtypes

import concourse.tile as tile
from concourse import bacc, mybir
from concourse.bass_utils import run_bass_kernel_spmd

F32 = mybir.dt.float32
BF16 = mybir.dt.bfloat16

S = 4096
D = 1024
HD = 64
N_CORES = 8
KT = 128
BQ = 512
CHUNK = S // N_CORES
NFT = D // 128
NKT = S // KT
NQB = S // BQ
G = 2

# stream_shuffle mask: swap 16-row halves within each 32-partition quadrant
SWAP16 = [16 + i for i in range(16)] + list(range(16))


def _build():
    nc = bacc.Bacc("TRN2", target_bir_lowering=False, debug=False, num_devices=N_CORES)

    xT = nc.dram_tensor("xT", [D, S], BF16, kind="ExternalInput")
    wq = nc.dram_tensor("wq", [D, 128], BF16, kind="ExternalInput")
    wk = nc.dram_tensor("wk", [D, 128], BF16, kind="ExternalInput")
    wv = nc.dram_tensor("wv", [D, 128], BF16, kind="ExternalInput")
    wo = nc.dram_tensor("wo", [D, D], BF16, kind="ExternalInput")
    cosP = nc.dram_tensor("cosP", [128, S], BF16, kind="ExternalInput")
    sinN = nc.dram_tensor("sinN", [128, S], BF16, kind="ExternalInput")
    lu = nc.dram_tensor("lu", [128, 128], BF16, kind="ExternalInput")
    out = nc.dram_tensor("out", [CHUNK, D], F32, kind="ExternalOutput")

    a2a_in = nc.dram_tensor("a2a_in", [N_CORES * 128, CHUNK], BF16)
    a2a_out = nc.dram_tensor("a2a_out", [N_CORES * 128, CHUNK], BF16)

    with tile.TileContext(nc) as tc, ExitStack() as ctx:
        sb = ctx.enter_context(tc.tile_pool(name="sb", bufs=1))
        xt_s = [sb.tile([128, S], BF16, tag=f"xt{t}", name=f"xt{t}") for t in range(NFT)]
        wq_s = sb.tile([128, NFT * 128], BF16, tag="wq", name="wq_s")
        wk_s = sb.tile([128, NFT * 128], BF16, tag="wk", name="wk_s")
        wv_s = sb.tile([128, NFT * 128], BF16, tag="wv", name="wv_s")
        wo_s = sb.tile([128, NFT, D], BF16, tag="wo", name="wo_s")
        cos_s = sb.tile([128, S], BF16, tag="cos", name="cos_s")
        sin_s = sb.tile([128, S], BF16, tag="sin", name="sin_s")
        lu_s = sb.tile([128, 128], BF16, tag="lu", name="lu_s")
        qA = sb.tile([128, S], BF16, tag="qA", name="qA")
        kA = sb.tile([128, S], BF16, tag="kA", name="kA")
        qT = sb.tile([128, S], BF16, tag="qT", name="qT")
        kT_ = sb.tile([128, S], BF16, tag="kT", name="kT_")
        v_aug = sb.tile([128, NKT, 2, 65], BF16, tag="vaug", name="v_aug")
        attnT = sb.tile([128, S], BF16, tag="attnT", name="attnT")
        aT = [sb.tile([128, CHUNK], BF16, tag=f"aT{t}", name=f"aT{t}") for t in range(NFT)]

        nc.vector.memset(v_aug[:], 1.0)
        warm_i = sb.tile([1, 64], F32, tag="warm_i", name="warm_i")
        warm_o = sb.tile([128, 64], F32, tag="warm_o", name="warm_o")
        nc.vector.memset(warm_i[:], 1.0)
        nc.gpsimd.partition_broadcast(warm_o[:], warm_i[:])

        nc.sync.dma_start(
            wq_s[:].rearrange("p (t c) -> p t c", t=NFT),
            wq[:, :].rearrange("(t p) c -> p t c", t=NFT),
        )
        nc.sync.dma_start(
            wk_s[:].rearrange("p (t c) -> p t c", t=NFT),
            wk[:, :].rearrange("(t p) c -> p t c", t=NFT),
        )
        nc.sync.dma_start(
            wv_s[:].rearrange("p (t c) -> p t c", t=NFT),
            wv[:, :].rearrange("(t p) c -> p t c", t=NFT),
        )
        nc.sync.dma_start(cos_s[:], cosP[:, :])
        nc.sync.dma_start(sin_s[:], sinN[:, :])
        nc.sync.dma_start(lu_s[:], lu[:, :])
        nc.sync.dma_start(
            wo_s[:], wo[:, :].rearrange("(t p) c -> p t c", t=NFT)
        )
        # x loads issued from the (otherwise idle) scalar queue, 1KB rows
        for nb2 in range(NQB // 2):
            c = slice(1024 * nb2, 1024 * (nb2 + 1))
            for t in range(NFT):
                nc.gpsimd.dma_start(xt_s[t][:, c], xT[128 * t : 128 * (t + 1), c])

        psc = ctx.enter_context(tc.tile_pool(name="psc", bufs=3, space="PSUM"))
        ppv = ctx.enter_context(tc.tile_pool(name="ppv", bufs=2, space="PSUM"))

        def proj_block(nb):
            c = slice(BQ * nb, BQ * (nb + 1))
            qp = psc.tile([128, BQ], F32, tag="sc", name="qp")
            for t in range(NFT):
                nc.tensor.matmul(
                    qp[:], wq_s[:, 128 * t : 128 * (t + 1)], xt_s[t][:, c],
                    start=(t == 0), stop=(t == NFT - 1),
                )
            kp = psc.tile([128, BQ], F32, tag="sc", name="kp")
            for t in range(NFT):
                nc.tensor.matmul(
                    kp[:], wk_s[:, 128 * t : 128 * (t + 1)], xt_s[t][:, c],
                    start=(t == 0), stop=(t == NFT - 1),
                )
            nc.vector.tensor_copy(qA[:, c], qp[:])
            nc.vector.tensor_copy(kA[:, c], kp[:])
            vp = psc.tile([128, BQ], F32, tag="sc", name="vp")
            for u in range(4):
                st = slice(BQ * nb + 128 * u, BQ * nb + 128 * (u + 1))
                for t in range(NFT):
                    nc.tensor.matmul(
                        vp[:, 128 * u : 128 * (u + 1)], xt_s[t][:, st],
                        wv_s[:, 128 * t : 128 * (t + 1)],
                        start=(t == 0), stop=(t == NFT - 1),
                    )
            nc.vector.tensor_copy(
                v_aug[:, 4 * nb : 4 * (nb + 1), :, 0:64],
                vp[:].rearrange("p (u h c) -> p u h c", u=4, h=2, c=64),
            )
            qBt = sb.tile([128, BQ], BF16, tag="qB", name="qBt", bufs=2)
            kBt = sb.tile([128, BQ], BF16, tag="kB", name="kBt", bufs=2)
            nc.vector.stream_shuffle(qBt[:], qA[:, c], SWAP16)
            nc.vector.stream_shuffle(kBt[:], kA[:, c], SWAP16)
            for A, B, R in ((qA, qBt, qT), (kA, kBt, kT_)):
                nc.vector.tensor_mul(R[:, c], A[:, c], cos_s[:, c])
                nc.vector.tensor_mul(B[:], B[:], sin_s[:, c])
                nc.vector.tensor_add(R[:, c], R[:, c], B[:])

        def attn_block(Q):
            q0 = BQ * Q
            n_jt = 4 * (Q + 1)
            jts = list(range(n_jt))
            groups = [jts[i : i + G] for i in range(0, n_jt, G)]
            seq = [(h, g) for g in groups for h in range(2)]
            outT = {}

            def emit_pv(h, g, ex):
                if g[0] == 0:
                    outT[h] = ppv.tile([65, BQ], F32, tag="pv", name=f"outT{h}")
                for idx, jt in enumerate(g):
                    trim = max(0, KT * jt - q0)
                    nc.tensor.matmul(
                        outT[h][:, trim:BQ],
                        v_aug[:, jt, h, :],
                        ex[:, BQ * idx + trim : BQ * (idx + 1)],
                        start=(jt == 0),
                        stop=(jt == n_jt - 1),
                    )

            pending = None
            for h, g in seq:
                hb = 64 * h
                sc = psc.tile([128, BQ * len(g)], F32, tag="sc", name="sc")
                for idx, jt in enumerate(g):
                    trim = max(0, KT * jt - q0)
                    nc.tensor.matmul(
                        sc[:, BQ * idx + trim : BQ * (idx + 1)],
                        kT_[hb : hb + 64, KT * jt : KT * (jt + 1)],
                        qT[hb : hb + 64, q0 + trim : q0 + BQ],
                        start=True, stop=True,
                    )
                for idx, jt in enumerate(g):
                    if KT * jt >= q0:
                        trim = KT * jt - q0
                        nc.vector.tensor_add(
                            sc[:, BQ * idx + trim : BQ * idx + trim + 128],
                            sc[:, BQ * idx + trim : BQ * idx + trim + 128],
                            lu_s[:],
                        )
                ex = sb.tile([128, G * BQ], BF16, tag="expT", name="expT", bufs=3)
                t0 = max(0, KT * g[0] - q0)
                nc.scalar.activation(
                    ex[:, t0 : BQ * len(g)], sc[:, t0 : BQ * len(g)],
                    mybir.ActivationFunctionType.Exp, scale=0.125,
                )
                if pending is not None:
                    emit_pv(*pending)
                pending = (h, g, ex)
            emit_pv(*pending)

            den_s = sb.tile([1, 2 * BQ], F32, tag="dens", name="dens", bufs=2)
            den_f = sb.tile([1, 2 * BQ], F32, tag="denf", name="denf", bufs=2)
            bc_sb = sb.tile([128, 2 * BQ], F32, tag="bc", name="bc", bufs=2)
            for h in range(2):
                hb = 64 * h
                nc.vector.tensor_copy(den_s[:, BQ * h : BQ * (h + 1)], outT[h][64:65, :])
                nc.vector.tensor_copy(attnT[hb : hb + 64, q0 : q0 + BQ], outT[h][0:64, :])
            nc.vector.reciprocal_approx_fast(den_f[:], den_s[:])
            nc.gpsimd.partition_broadcast(bc_sb[:], den_f[:])
            for h in range(2):
                hb = 64 * h
                nc.vector.tensor_mul(
                    attnT[hb : hb + 64, q0 : q0 + BQ],
                    attnT[hb : hb + 64, q0 : q0 + BQ],
                    bc_sb[hb : hb + 64, BQ * h : BQ * (h + 1)],
                )
            nc.sync.dma_start(
                a2a_in[128 * Q : 128 * (Q + 1), :], attnT[:, q0 : q0 + BQ]
            )

        for nb in range(NQB):
            proj_block(nb)
            if nb >= 1:
                attn_block(nb - 1)
        attn_block(NQB - 1)

        nc.gpsimd.collective_compute(
            "AllToAll",
            mybir.AluOpType.bypass,
            replica_groups=[list(range(N_CORES))],
            ins=[a2a_in.ap().opt()],
            outs=[a2a_out.ap().opt()],
        )
        for t in range(NFT):
            nc.sync.dma_start(aT[t][:], a2a_out[128 * t : 128 * (t + 1), :])

        for it in range(CHUNK // 128):
            for oh in range(D // 512):
                p = psc.tile([128, 512], F32, tag="sc", name="p_o")
                for t in range(NFT):
                    nc.tensor.matmul(
                        p[:],
                        aT[t][:, 128 * it : 128 * (it + 1)],
                        wo_s[:, t, 512 * oh : 512 * (oh + 1)],
                        start=(t == 0), stop=(t == NFT - 1),
                    )
                ot = sb.tile([128, 512], F32, tag="oflush", name="ot", bufs=2)
                nc.scalar.copy(ot[:], p[:])
                nc.sync.dma_start(
                    out[128 * it : 128 * (it + 1), 512 * oh : 512 * (oh + 1)], ot[:]
                )

    nc.compile()
    return nc


def _host_prep(x, Wq, Wk, Wv, Wo):
    bf = ml_dtypes.bfloat16
    # quadrant-local RoPE layout: within each head (64 rows = 2 quadrants of
    # 32), quadrant q holds pairs [16q:16q+16): rows 32q+j = dim 2(16q+j)
    # (even), rows 32q+16+j = dim 2(16q+j)+1 (odd). The rotary partner of a
    # row is then 16 rows away inside the same quadrant (stream_shuffle).
    perm = np.empty(HD, dtype=np.int64)
    pair = np.empty(HD, dtype=np.int64)
    sign = np.empty(HD, dtype=np.float32)
    for q in range(2):
        for j in range(16):
            perm[32 * q + j] = 2 * (16 * q + j)
            perm[32 * q + 16 + j] = 2 * (16 * q + j) + 1
            pair[32 * q + j] = 16 * q + j
            pair[32 * q + 16 + j] = 16 * q + j
            sign[32 * q + j] = -1.0
            sign[32 * q + 16 + j] = 1.0

    inv_freq = 1.0 / (10000.0 ** (np.arange(0, HD, 2, dtype=np.float32) / HD))
    fr = np.outer(np.arange(S, dtype=np.float32), inv_freq)
    cosA = np.cos(fr).T  # [32, S] per pair index
    sinA = np.sin(fr).T
    cosH = cosA[pair]                      # [64, S]
    sinH = sinA[pair] * sign[:, None]      # [64, S]
    cosP = np.tile(cosH, (2, 1)).astype(bf)
    sinN = np.tile(sinH, (2, 1)).astype(bf)
    lu = np.tril(np.full((128, 128), -400.0, np.float32), k=-1).astype(bf)

    xT = np.ascontiguousarray(x.reshape(S, D).T).astype(bf)
    woT = np.ascontiguousarray(np.asarray(Wo, np.float32).T).astype(bf)

    in_maps = []
    for c in range(N_CORES):
        rows = np.concatenate([128 * c + 64 * h + perm for h in range(2)])
        in_maps.append(
            {
                "xT": xT,
                "wq": np.ascontiguousarray(np.asarray(Wq, np.float32)[rows].T).astype(bf),
                "wk": np.ascontiguousarray(np.asarray(Wk, np.float32)[rows].T).astype(bf),
                "wv": np.ascontiguousarray(
                    np.asarray(Wv, np.float32)[128 * c : 128 * (c + 1)].T
                ).astype(bf),
                "wo": woT,
                "cosP": cosP,
                "sinN": sinN,
                "lu": lu,
            }
        )
    return in_maps


_NC_CACHE = None


def kernel(x, Wq, Wk, Wv, Wo):
    global _NC_CACHE
    if _NC_CACHE is None:
        _NC_CACHE = _build()
    nc = _NC_CACHE
    in_maps = _host_prep(
        np.asarray(x, np.float32),
        np.asarray(Wq, np.float32),
        np.asarray(Wk, np.float32),
        np.asarray(Wv, np.float32),
        np.asarray(Wo, np.float32),
    )
    res = run_bass_kernel_spmd(nc, in_maps, core_ids=list(range(N_CORES)))
    full = np.concatenate([res.results[c]["out"] for c in range(N_CORES)], axis=0)
    return full.reshape(1, S, D).astype(np.float32)
